# revision 9
# baseline (speedup 1.0000x reference)
"""Bass/Trainium2 kernel for the span bag-of-words (multi-hot) + Linear problem.

Reference semantics (B=16, S=64, L=1024, V=50000, D=512):
    bow[b,s,v] = 1 if v occurs in input_ids[b, i:j] for (i,j)=span_idxs[b,s]
    out[b,s,:] = bow[b,s,:] @ W.T + bias            # [B,S,D]

Key identity used here: a position t contributes W[:, ids[t]] to span (i,j)
iff i <= t < j AND prev[t] < i, where prev[t] is the index of the previous
occurrence of ids[t] in the same batch row (-1 if none).  This turns the
dense [B*S, V] x [V, D] matmul (52 GFLOP + 102MB of W traffic) into:
  - a gather of the 1024 referenced W columns per batch row (dma_gather),
  - an O(L^2) pairwise-equality pass to get prev[] (vector engine),
  - a [S, L] x [L, D] masked matmul per batch row (134 MFLOP total).

Sharding: data-parallel over batch. 8 cores x 2 batch rows each. No
collectives; each core writes its own output slice.

Implementation notes (hard-won):
  * walrus codegen allows only ONE sync-wait per instruction (EventSemaphore
    holds 2).  Bacc.compile() legalizes, but TensorScalarPtr-heavy code keeps
    fewer waits if operands that feed one op arrive via a single DMA, so
    inputs are packed into few DRAM tensors and elementwise ops are pinned to
    the vector engine (intra-engine program order covers most deps).
  * dma_gather's Q7 ucode runs on an rx/tx core pair; BOTH cores stream the
    int16 index list, rx from partitions 0-15, tx from partitions 16-31, so
    the index tile must carry two copies.  (The CoreSim model reads only
    partitions 0-15 - HW needs rows 16-31 too.)
  * dma_gather indices are int16 (sign-extended): table rows must be <32768.
    W^T is split into two 25001-row tables (each with a zero row appended);
    out-of-half slots gather the zero row and both partial E streams are
    accumulated in the PSUM matmul, which costs PE time but no vector ops.
"""

import os
import sys

import numpy as np

for _p in ("/opt/trn_rl_repo", "/root/.axon_site/_ro/trn_rl_repo"):
    if os.path.isdir(_p) and _p not in sys.path:
        sys.path.append(_p)

import concourse.bacc as bacc
import concourse.bass as bass
import concourse.mybir as mybir
import concourse.tile as tile
from concourse.bass_utils import run_bass_kernel_spmd

P = 128          # partitions
B, S, L, V, D = 16, 64, 1024, 50000, 512
NCORES = 8
NB = B // NCORES     # batch rows per core = 2
NCH = L // P         # 128-position chunks per batch row = 8
VH = V // 2          # vocab half = 25000
VT = VH + 1          # table rows incl. zero row
MVW = P * (NCH * (NCH + 1) // 2)   # total maskval width = 4608
MV1 = (1 + 2 + 3 + 4) * P          # first-half maskval width = 1280
CSTW = MVW + 2 * P + NCH           # mv | ij(256) | tpos(8) = 4872
IDSW = NB * L + NB * NCH           # idsb0 | idsb1 | idscf = 2064
NCI = L // 16        # idx columns per gather list = 64

AL = mybir.AluOpType
F32 = mybir.dt.float32
I16 = mybir.dt.int16


def _build_program():
    nc = bacc.Bacc("TRN2", target_bir_lowering=False, debug=False,
                   num_devices=NCORES)

    wtlo = nc.dram_tensor("wtlo", [VT, D], F32, kind="ExternalInput").ap()
    wthi = nc.dram_tensor("wthi", [VT, D], F32, kind="ExternalInput").ap()
    idsall = nc.dram_tensor("idsall", [P, IDSW], F32, kind="ExternalInput").ap()
    # int16 gather lists: [b0lo | b0hi | b1lo | b1hi], rows 0-15 = rows 16-31
    idx16 = nc.dram_tensor("idx16", [P, NB * 2 * NCI], I16,
                           kind="ExternalInput").ap()
    cst = nc.dram_tensor("cst", [P, CSTW], F32, kind="ExternalInput").ap()
    bias = nc.dram_tensor("bias", [D], F32, kind="ExternalInput").ap()
    out = nc.dram_tensor("out", [NB, S, D], F32, kind="ExternalOutput").ap()

    with tile.TileContext(nc) as tc:
        with (
            tc.tile_pool(name="const", bufs=1) as cp,
            tc.tile_pool(name="work", bufs=2) as wp,
            tc.tile_pool(name="psum", bufs=1, space="PSUM") as pp,
        ):
            # ---- input loads ----
            ids_sb = cp.tile([P, IDSW], F32, tag="idsall")
            nc.sync.dma_start(out=ids_sb[:], in_=idsall)
            idx_sb = cp.tile([P, NB * 2 * NCI], I16, tag="idx16")
            nc.sync.dma_start(out=idx_sb[:], in_=idx16)
            cst_sb = cp.tile([P, CSTW], F32, tag="cst")
            # split so ij/tpos and the first maskval blocks land early
            nc.sync.dma_start(out=cst_sb[:, MVW:], in_=cst[:, MVW:])
            nc.sync.dma_start(out=cst_sb[:, :MV1], in_=cst[:, :MV1])
            nc.sync.dma_start(out=cst_sb[:, MV1:MVW], in_=cst[:, MV1:MVW])
            bias_sb = cp.tile([1, D], F32, tag="bias")
            nc.sync.dma_start(out=bias_sb[:], in_=bias.unsqueeze(0))
            ones_sb = cp.tile([1, S], F32, tag="ones")
            nc.vector.memset(ones_sb[:], 1.0)

            def idsb(b):          # [P, L] ids of batch row b, partition-bcast
                return ids_sb[:, b * L:(b + 1) * L]

            def idscf(k):         # [P, 1] per-partition token col k = b*NCH+c
                return ids_sb[:, NB * L + k: NB * L + k + 1]

            ij_sb = cst_sb[:, MVW:MVW + 2 * P]          # [P, 256]
            tpos_sb = cst_sb[:, MVW + 2 * P:]           # [P, 8]

            # ---- gather E[t, :] = WT[ids[t], :], slot t -> [t%128, t//128, :]
            e_t = []      # e_t[b][h] = [P, NCH*D] tile
            for b in range(NB):
                ebs = []
                for h, wsrc in enumerate((wtlo, wthi)):
                    eb = cp.tile([P, NCH * D], F32, tag=f"e{b}{h}")
                    nc.gpsimd.dma_gather(
                        eb[:].rearrange("p (c d) -> p c d", d=D),
                        wsrc,
                        idx_sb[:, (b * 2 + h) * NCI:(b * 2 + h + 1) * NCI],
                        L, L, D)
                    ebs.append(eb)
                e_t.append(ebs)

            # ---- prev1[t] = 1 + index of previous occurrence of ids[t] (0 if none)
            # chunk c holds t = c*128 + p in partition p, column c.
            prev1 = []
            for b in range(NB):
                pb = cp.tile([P, NCH], F32, tag=f"prev{b}")
                for c in range(NCH):
                    F = (c + 1) * P
                    off = (c * (c + 1) // 2) * P
                    eq = wp.tile([P, L], F32, tag="eq")
                    if b == 0 and c in (0, 4):
                        # absorb the maskval DMA-completion tick on a spare op
                        # (1-wait budget); WAW on eq orders it before the TS
                        src = 0 if c == 0 else MV1
                        nc.vector.tensor_tensor(out=eq[:1, :1],
                                                in0=cst_sb[:1, src:src + 1],
                                                in1=cst_sb[:1, src:src + 1],
                                                op=AL.add)
                    nc.vector.tensor_scalar(
                        out=eq[:, :F], in0=idsb(b)[:, :F],
                        scalar1=idscf(b * NCH + c), scalar2=None,
                        op0=AL.is_equal)
                    # NB: tensor_tensor_reduce would fuse these, but its
                    # extended-ISA ucode wedges the device on this runtime.
                    scr = wp.tile([P, L], F32, tag="scr")
                    nc.vector.tensor_tensor(
                        out=scr[:, :F], in0=eq[:, :F],
                        in1=cst_sb[:, off:off + F], op=AL.mult)
                    nc.vector.tensor_reduce(
                        out=pb[:, c:c + 1], in_=scr[:, :F],
                        axis=mybir.AxisListType.XYZW, op=AL.max)
                prev1.append(pb)

            # ---- matmul accumulation: psum[b] = bias + sum_c Mt_c[b].T @ E_c[b]
            psums = []
            for b in range(NB):
                ps = pp.tile([S, D], F32, tag=f"ps{b}")
                nc.tensor.matmul(out=ps[:], lhsT=ones_sb[:], rhs=bias_sb[:],
                                 start=True, stop=False)
                psums.append(ps)

            # mask chunk Mt_c[p, f] for span f = b*64+s, position t = c*128+p:
            #   (i_f <= t) & (t < j_f) & (i_f >= prev1_b[t])
            # computed as (a - d) * g with a=[i<=t], d=[i<prev1] (d subset a),
            # g=[j>t].
            for c in range(NCH):
                a_t = wp.tile([P, P], F32, tag="a")
                nc.vector.tensor_scalar(out=a_t[:], in0=ij_sb[:, :P],
                                        scalar1=tpos_sb[:, c:c + 1],
                                        scalar2=None, op0=AL.is_le)
                g_t = wp.tile([P, P], F32, tag="g")
                nc.vector.tensor_scalar(out=g_t[:], in0=ij_sb[:, P:2 * P],
                                        scalar1=tpos_sb[:, c:c + 1],
                                        scalar2=None, op0=AL.is_gt)
                d_t = wp.tile([P, P], F32, tag="d")
                for b in range(NB):
                    nc.vector.tensor_scalar(out=d_t[:, b * S:(b + 1) * S],
                                            in0=ij_sb[:, b * S:b * S + S],
                                            scalar1=prev1[b][:, c:c + 1],
                                            scalar2=None, op0=AL.is_lt)
                u_t = wp.tile([P, P], F32, tag="u")
                nc.vector.tensor_tensor(out=u_t[:], in0=a_t[:], in1=d_t[:],
                                        op=AL.subtract)
                m_t = wp.tile([P, P], F32, tag="m")
                nc.vector.tensor_tensor(out=m_t[:], in0=u_t[:], in1=g_t[:],
                                        op=AL.mult)
                for b in range(NB):
                    for h in range(2):
                        nc.tensor.matmul(out=psums[b][:],
                                         lhsT=m_t[:, b * S:(b + 1) * S],
                                         rhs=e_t[b][h][:, c * D:(c + 1) * D],
                                         start=False,
                                         stop=(c == NCH - 1 and h == 1))

            # ---- write out ----
            for b in range(NB):
                o_sb = wp.tile([S, D], F32, tag=f"o{b}")
                nc.vector.tensor_copy(out=o_sb[:], in_=psums[b][:])
                nc.sync.dma_start(out=out[b], in_=o_sb[:])

    # bacc passes: split excess sync waits into EventSemaphore insts,
    # move matmul waits to ldweights, populate extended-inst ISA bytes, etc.
    nc.compile()
    return nc


_NC_CACHE = None


def _get_program():
    global _NC_CACHE
    if _NC_CACHE is None:
        _NC_CACHE = _build_program()
    return _NC_CACHE


def _host_constants():
    # maskval blocks: for chunk c (t = c*128+p), source positions f in
    # [0, (c+1)*128): value f+1 if f < t else -1e9 (ignored by max).
    cstw = np.empty((P, CSTW), np.float32)
    f_idx = np.arange(L, dtype=np.float32)
    for c in range(NCH):
        F = (c + 1) * P
        off = (c * (c + 1) // 2) * P
        t = (c * P + np.arange(P, dtype=np.float32))[:, None]   # [P,1]
        cstw[:, off:off + F] = np.where(f_idx[None, :F] < t,
                                        f_idx[None, :F] + 1.0,
                                        np.float32(-1e9))
    tpos = (np.arange(NCH, dtype=np.float32)[None, :] * P
            + np.arange(P, dtype=np.float32)[:, None])          # [P, NCH]
    cstw[:, MVW + 2 * P:] = tpos
    return cstw


def _gather_list(ids_row, lo):
    """int16 index list [P, NCI] for one batch row and one vocab half.
    Slot t lives at [t % 16, t // 16]; rows 16-31 duplicate rows 0-15
    (tx Q7 core reads them); out-of-half slots point at the zero row VH."""
    if lo:
        idx = np.where(ids_row < VH, ids_row, VH)
    else:
        idx = np.where(ids_row >= VH, ids_row - VH, VH)
    lst = np.zeros((P, NCI), np.int16)
    lst[:16] = idx.reshape(NCI, 16).T.astype(np.int16)
    lst[16:32] = lst[:16]
    return lst


def _make_in_maps(input_ids, span_idxs, W, b):
    ids = np.asarray(input_ids).astype(np.int64)        # [B, L]
    spans = np.asarray(span_idxs).astype(np.int64)      # [B, S, 2]
    Wf = np.asarray(W, dtype=np.float32)                # [D, V]
    WT = np.ascontiguousarray(Wf.T)                     # [V, D]
    zrow = np.zeros((1, D), np.float32)
    wtlo = np.ascontiguousarray(np.vstack([WT[:VH], zrow]))
    wthi = np.ascontiguousarray(np.vstack([WT[VH:], zrow]))
    bf = np.ascontiguousarray(np.asarray(b, dtype=np.float32))  # [D]
    cst_base = _host_constants()

    in_maps = []
    for core in range(NCORES):
        sl = slice(NB * core, NB * (core + 1))
        ids_c = ids[sl]                                 # [NB, L]
        sp = spans[sl]                                  # [NB, S, 2]
        # column-chunk layout: [p, b*NCH + c] = ids_c[b, c*128 + p]
        idsc = np.ascontiguousarray(
            ids_c.reshape(NB, NCH, P).transpose(2, 0, 1).reshape(P, NB * NCH))
        idsall = np.empty((P, IDSW), np.float32)
        for bb in range(NB):
            idsall[:, bb * L:(bb + 1) * L] = ids_c[bb][None, :]
        idsall[:, NB * L:] = idsc
        idx16 = np.concatenate(
            [_gather_list(ids_c[bb], lo) for bb in range(NB)
             for lo in (True, False)], axis=1)
        cst = cst_base.copy()
        ij = np.concatenate([sp[..., 0].reshape(-1),
                             sp[..., 1].reshape(-1)]).astype(np.float32)
        cst[:, MVW:MVW + 2 * P] = ij[None, :]
        in_maps.append({
            "wtlo": wtlo,
            "wthi": wthi,
            "idsall": idsall,
            "idx16": np.ascontiguousarray(idx16),
            "cst": cst,
            "bias": bf,
        })
    return in_maps


def run(input_ids, span_idxs, W, b, trace=False, **spmd_kwargs):
    """Build + run on 8 cores; returns (out [B,S,D] f32, BassKernelResults)."""
    nc = _get_program()
    in_maps = _make_in_maps(input_ids, span_idxs, W, b)
    res = run_bass_kernel_spmd(nc, in_maps, list(range(NCORES)),
                               trace=trace, **spmd_kwargs)
    outs = [res.results[i]["out"] for i in range(NCORES)]
    full = np.concatenate(outs, axis=0).reshape(B, S, D).astype(np.float32)
    return full, res


def kernel(input_ids, span_idxs, W, b):
    out, _ = run(input_ids, span_idxs, W, b)
    return out


# revision 11
# speedup vs baseline: 1.3374x; 1.3374x over previous
"""Bass/Trainium2 kernel for the span bag-of-words (multi-hot) + Linear problem.

Reference semantics (B=16, S=64, L=1024, V=50000, D=512):
    bow[b,s,v] = 1 if v occurs in input_ids[b, i:j] for (i,j)=span_idxs[b,s]
    out[b,s,:] = bow[b,s,:] @ W.T + bias            # [B,S,D]

Key identity used here: a position t contributes W[:, ids[t]] to span (i,j)
iff i <= t < j AND prev[t] < i, where prev[t] is the index of the previous
occurrence of ids[t] in the same batch row (-1 if none).  This turns the
dense [B*S, V] x [V, D] matmul (52 GFLOP + 102MB of W traffic) into:
  - a gather of the 1024 referenced W columns per batch row (dma_gather),
  - an O(L^2) pairwise-equality pass to get prev[] (vector engine),
  - a [S, L] x [L, D] masked matmul per batch row (134 MFLOP total).

Sharding: data-parallel over batch. 8 cores x 2 batch rows each. No
collectives; each core writes its own output slice.

Implementation notes (hard-won):
  * walrus codegen allows only ONE sync-wait per instruction; Bacc.compile()
    legalizes by splitting into EventSemaphore instructions.  Inputs that
    feed one TensorScalarPtr arrive via a single DMA to keep wait counts low.
  * tensor_tensor_reduce (extended-ISA) wedges the device on this runtime -
    use tensor_tensor + tensor_reduce instead.
  * dma_gather's Q7 ucode runs on a core pair per SWDGE queue; BOTH cores
    stream the int16 index list: for queue q, rx reads partitions
    [32q, 32q+16) and tx reads [32q+16, 32q+32), so the list is duplicated
    in that 32-partition window.  (CoreSim models only partitions 0-15 and
    queue 0.)  Descriptor generation is ~8.7us per 1024-slot gather on one
    Q7 pair, so the 4 gathers run on 4 different queues in parallel.
  * dma_gather indices are int16 (sign-extended): table rows must be <32768.
    W^T is split into two 25001-row tables (each with a zero row appended);
    out-of-half slots gather the zero row and both partial E streams are
    accumulated in the PSUM matmul (extra PE work, no vector ops).
  * fp32 matmuls lower to 4 PE passes; gather tables / mask / matmuls run in
    bf16 (mask is exact 0/1; PSUM accumulation stays fp32).
"""

import os
import sys

import numpy as np

for _p in ("/opt/trn_rl_repo", "/root/.axon_site/_ro/trn_rl_repo"):
    if os.path.isdir(_p) and _p not in sys.path:
        sys.path.append(_p)

import concourse.bacc as bacc
import concourse.bass as bass
import concourse.mybir as mybir
import concourse.tile as tile
from concourse.bass_utils import run_bass_kernel_spmd

P = 128          # partitions
B, S, L, V, D = 16, 64, 1024, 50000, 512
NCORES = 8
NB = B // NCORES     # batch rows per core = 2
NCH = L // P         # 128-position chunks per batch row = 8
VH = V // 2          # vocab half = 25000
VT = VH + 1          # table rows incl. zero row
MVW = P * (NCH * (NCH + 1) // 2)   # total maskval width = 4608
MV1 = (1 + 2 + 3 + 4) * P          # first-half maskval width = 1280
CSTW = MVW + 2 * P + NCH           # mv | ij(256) | tpos(8) = 4872
IDSW = NB * L + NB * NCH           # idsb0 | idsb1 | idscf = 2064
NCI = L // 16        # idx columns per gather list = 64
NG = NB * 2          # gathers per core = 4

AL = mybir.AluOpType
F32 = mybir.dt.float32
BF16 = mybir.dt.bfloat16
I16 = mybir.dt.int16


def _build_program(sim_compat=False):
    nc = bacc.Bacc("TRN2", target_bir_lowering=False, debug=False,
                   num_devices=NCORES, num_swdge_queues=1 if sim_compat else 4)

    wtlo = nc.dram_tensor("wtlo", [VT, D], BF16, kind="ExternalInput").ap()
    wthi = nc.dram_tensor("wthi", [VT, D], BF16, kind="ExternalInput").ap()
    idsall = nc.dram_tensor("idsall", [P, IDSW], F32, kind="ExternalInput").ap()
    # int16 gather lists, one [P, NCI] block per gather g = b*2+h; on HW the
    # list for gather g sits in partition rows [32g, 32g+32) of its block
    idx16 = nc.dram_tensor("idx16", [P, NG * NCI], I16,
                           kind="ExternalInput").ap()
    cst = nc.dram_tensor("cst", [P, CSTW], F32, kind="ExternalInput").ap()
    bias = nc.dram_tensor("bias", [D], F32, kind="ExternalInput").ap()
    out = nc.dram_tensor("out", [NB, S, D], F32, kind="ExternalOutput").ap()

    with tile.TileContext(nc) as tc:
        with (
            tc.tile_pool(name="const", bufs=1) as cp,
            tc.tile_pool(name="work", bufs=2) as wp,
            tc.tile_pool(name="psum", bufs=1, space="PSUM") as pp,
        ):
            # ---- input loads ----
            ids_sb = cp.tile([P, IDSW], F32, tag="idsall")
            nc.sync.dma_start(out=ids_sb[:], in_=idsall)
            idx_sb = cp.tile([P, NG * NCI], I16, tag="idx16")
            nc.sync.dma_start(out=idx_sb[:], in_=idx16)
            cst_sb = cp.tile([P, CSTW], F32, tag="cst")
            # split so ij/tpos and the first maskval blocks land early
            nc.sync.dma_start(out=cst_sb[:, MVW:], in_=cst[:, MVW:])
            nc.sync.dma_start(out=cst_sb[:, :MV1], in_=cst[:, :MV1])
            nc.sync.dma_start(out=cst_sb[:, MV1:MVW], in_=cst[:, MV1:MVW])
            bias_sb = cp.tile([1, D], F32, tag="bias")
            nc.sync.dma_start(out=bias_sb[:], in_=bias.unsqueeze(0))
            ones_sb = cp.tile([1, S], F32, tag="ones")
            nc.vector.memset(ones_sb[:], 1.0)

            def idsb(b):          # [P, L] ids of batch row b, partition-bcast
                return ids_sb[:, b * L:(b + 1) * L]

            def idscf(k):         # [P, 1] per-partition token col k = b*NCH+c
                return ids_sb[:, NB * L + k: NB * L + k + 1]

            ij_sb = cst_sb[:, MVW:MVW + 2 * P]          # [P, 256]
            tpos_sb = cst_sb[:, MVW + 2 * P:]           # [P, 8]

            # ---- gather E[t, :] = WT[ids[t], :], slot t -> [t%128, t//128, :]
            e_t = []      # e_t[b][h] = [P, NCH*D] bf16 tile
            for b in range(NB):
                ebs = []
                for h, wsrc in enumerate((wtlo, wthi)):
                    g = b * 2 + h
                    eb = cp.tile([P, NCH * D], BF16, tag=f"e{b}{h}")
                    nc.gpsimd.dma_gather(
                        eb[:].rearrange("p (c d) -> p c d", d=D),
                        wsrc,
                        idx_sb[:, g * NCI:(g + 1) * NCI],
                        L, L, D,
                        queue_num=0 if sim_compat else g)
                    ebs.append(eb)
                e_t.append(ebs)

            # ---- prev1[t] = 1 + index of previous occurrence of ids[t] (0 if none)
            # chunk c holds t = c*128 + p in partition p, column c.
            prev1 = []
            for b in range(NB):
                pb = cp.tile([P, NCH], F32, tag=f"prev{b}")
                for c in range(NCH):
                    F = (c + 1) * P
                    off = (c * (c + 1) // 2) * P
                    eq = wp.tile([P, L], F32, tag="eq")
                    if b == 0 and c in (0, 4):
                        # absorb the maskval DMA-completion tick on a spare op
                        # (1-wait budget); WAW on eq orders it before the TS
                        src = 0 if c == 0 else MV1
                        nc.vector.tensor_tensor(out=eq[:1, :1],
                                                in0=cst_sb[:1, src:src + 1],
                                                in1=cst_sb[:1, src:src + 1],
                                                op=AL.add)
                    nc.vector.tensor_scalar(
                        out=eq[:, :F], in0=idsb(b)[:, :F],
                        scalar1=idscf(b * NCH + c), scalar2=None,
                        op0=AL.is_equal)
                    # NB: tensor_tensor_reduce would fuse these, but its
                    # extended-ISA ucode wedges the device on this runtime.
                    scr = wp.tile([P, L], F32, tag="scr")
                    nc.vector.tensor_tensor(
                        out=scr[:, :F], in0=eq[:, :F],
                        in1=cst_sb[:, off:off + F], op=AL.mult)
                    nc.vector.tensor_reduce(
                        out=pb[:, c:c + 1], in_=scr[:, :F],
                        axis=mybir.AxisListType.X, op=AL.max)
                prev1.append(pb)

            # ---- matmul accumulation: psum[b] = bias + sum_c Mt_c[b].T @ E_c[b]
            psums = []
            for b in range(NB):
                ps = pp.tile([S, D], F32, tag=f"ps{b}")
                nc.tensor.matmul(out=ps[:], lhsT=ones_sb[:], rhs=bias_sb[:],
                                 start=True, stop=False)
                psums.append(ps)

            # mask chunk Mt_c[p, f] for span f = b*64+s, position t = c*128+p:
            #   (i_f <= t) & (t < j_f) & (i_f >= prev1_b[t])
            # computed as (a - d) * g with a=[i<=t], d=[i<prev1] (d subset a),
            # g=[j>t].
            for c in range(NCH):
                a_t = wp.tile([P, P], F32, tag="a")
                nc.vector.tensor_scalar(out=a_t[:], in0=ij_sb[:, :P],
                                        scalar1=tpos_sb[:, c:c + 1],
                                        scalar2=None, op0=AL.is_le)
                g_t = wp.tile([P, P], F32, tag="g")
                nc.vector.tensor_scalar(out=g_t[:], in0=ij_sb[:, P:2 * P],
                                        scalar1=tpos_sb[:, c:c + 1],
                                        scalar2=None, op0=AL.is_gt)
                d_t = wp.tile([P, P], F32, tag="d")
                for b in range(NB):
                    nc.vector.tensor_scalar(out=d_t[:, b * S:(b + 1) * S],
                                            in0=ij_sb[:, b * S:b * S + S],
                                            scalar1=prev1[b][:, c:c + 1],
                                            scalar2=None, op0=AL.is_lt)
                u_t = wp.tile([P, P], F32, tag="u")
                nc.vector.tensor_tensor(out=u_t[:], in0=a_t[:], in1=d_t[:],
                                        op=AL.subtract)
                m_t = wp.tile([P, P], BF16, tag="m")
                nc.vector.tensor_tensor(out=m_t[:], in0=u_t[:], in1=g_t[:],
                                        op=AL.mult)
                for b in range(NB):
                    for h in range(2):
                        nc.tensor.matmul(out=psums[b][:],
                                         lhsT=m_t[:, b * S:(b + 1) * S],
                                         rhs=e_t[b][h][:, c * D:(c + 1) * D],
                                         start=False,
                                         stop=(c == NCH - 1 and h == 1))

            # ---- write out ----
            for b in range(NB):
                o_sb = wp.tile([S, D], F32, tag=f"o{b}")
                nc.vector.tensor_copy(out=o_sb[:], in_=psums[b][:])
                nc.sync.dma_start(out=out[b], in_=o_sb[:])

    # bacc passes: split excess sync waits into EventSemaphore insts,
    # move matmul waits to ldweights, populate extended-inst ISA bytes, etc.
    nc.compile()
    return nc


_NC_CACHE = {}


def _get_program(sim_compat=False):
    if sim_compat not in _NC_CACHE:
        _NC_CACHE[sim_compat] = _build_program(sim_compat)
    return _NC_CACHE[sim_compat]


def _host_constants():
    # maskval blocks: for chunk c (t = c*128+p), source positions f in
    # [0, (c+1)*128): value f+1 if f < t else -1e9 (ignored by max).
    cstw = np.empty((P, CSTW), np.float32)
    f_idx = np.arange(L, dtype=np.float32)
    for c in range(NCH):
        F = (c + 1) * P
        off = (c * (c + 1) // 2) * P
        t = (c * P + np.arange(P, dtype=np.float32))[:, None]   # [P,1]
        cstw[:, off:off + F] = np.where(f_idx[None, :F] < t,
                                        f_idx[None, :F] + 1.0,
                                        np.float32(-1e9))
    tpos = (np.arange(NCH, dtype=np.float32)[None, :] * P
            + np.arange(P, dtype=np.float32)[:, None])          # [P, NCH]
    cstw[:, MVW + 2 * P:] = tpos
    return cstw


def _gather_list(ids_row, lo):
    """int16 index list [16, NCI] for one batch row and one vocab half.
    Slot t lives at [t % 16, t // 16]; out-of-half slots point at the zero
    row VH."""
    if lo:
        idx = np.where(ids_row < VH, ids_row, VH)
    else:
        idx = np.where(ids_row >= VH, ids_row - VH, VH)
    return idx.reshape(NCI, 16).T.astype(np.int16)


def _make_in_maps(input_ids, span_idxs, W, b, sim_compat=False):
    import ml_dtypes
    ids = np.asarray(input_ids).astype(np.int64)        # [B, L]
    spans = np.asarray(span_idxs).astype(np.int64)      # [B, S, 2]
    Wf = np.asarray(W, dtype=np.float32)                # [D, V]
    WT = np.ascontiguousarray(Wf.T)                     # [V, D]
    zrow = np.zeros((1, D), np.float32)
    wtlo = np.ascontiguousarray(
        np.vstack([WT[:VH], zrow]).astype(ml_dtypes.bfloat16))
    wthi = np.ascontiguousarray(
        np.vstack([WT[VH:], zrow]).astype(ml_dtypes.bfloat16))
    bf = np.ascontiguousarray(np.asarray(b, dtype=np.float32))  # [D]
    cst_base = _host_constants()

    in_maps = []
    for core in range(NCORES):
        sl = slice(NB * core, NB * (core + 1))
        ids_c = ids[sl]                                 # [NB, L]
        sp = spans[sl]                                  # [NB, S, 2]
        # column-chunk layout: [p, b*NCH + c] = ids_c[b, c*128 + p]
        idsc = np.ascontiguousarray(
            ids_c.reshape(NB, NCH, P).transpose(2, 0, 1).reshape(P, NB * NCH))
        idsall = np.empty((P, IDSW), np.float32)
        for bb in range(NB):
            idsall[:, bb * L:(bb + 1) * L] = ids_c[bb][None, :]
        idsall[:, NB * L:] = idsc
        idx16 = np.zeros((P, NG * NCI), np.int16)
        for bb in range(NB):
            for h in range(2):
                g = bb * 2 + h
                lst = _gather_list(ids_c[bb], h == 0)   # [16, NCI]
                base = 0 if sim_compat else 32 * g
                idx16[base:base + 16, g * NCI:(g + 1) * NCI] = lst
                idx16[base + 16:base + 32, g * NCI:(g + 1) * NCI] = lst
        cst = cst_base.copy()
        ij = np.concatenate([sp[..., 0].reshape(-1),
                             sp[..., 1].reshape(-1)]).astype(np.float32)
        cst[:, MVW:MVW + 2 * P] = ij[None, :]
        in_maps.append({
            "wtlo": wtlo,
            "wthi": wthi,
            "idsall": idsall,
            "idx16": np.ascontiguousarray(idx16),
            "cst": cst,
            "bias": bf,
        })
    return in_maps


def run(input_ids, span_idxs, W, b, trace=False, **spmd_kwargs):
    """Build + run on 8 cores; returns (out [B,S,D] f32, BassKernelResults)."""
    nc = _get_program()
    in_maps = _make_in_maps(input_ids, span_idxs, W, b)
    res = run_bass_kernel_spmd(nc, in_maps, list(range(NCORES)),
                               trace=trace, **spmd_kwargs)
    outs = [res.results[i]["out"] for i in range(NCORES)]
    full = np.concatenate(outs, axis=0).reshape(B, S, D).astype(np.float32)
    return full, res


def kernel(input_ids, span_idxs, W, b):
    out, _ = run(input_ids, span_idxs, W, b)
    return out


# revision 14
# speedup vs baseline: 1.7500x; 1.3085x over previous
"""Bass/Trainium2 kernel for the span bag-of-words (multi-hot) + Linear problem.

Reference semantics (B=16, S=64, L=1024, V=50000, D=512):
    bow[b,s,v] = 1 if v occurs in input_ids[b, i:j] for (i,j)=span_idxs[b,s]
    out[b,s,:] = bow[b,s,:] @ W.T + bias            # [B,S,D]

Key identity used here: a position t contributes W[:, ids[t]] to span (i,j)
iff i <= t < j AND prev[t] < i, where prev[t] is the index of the previous
occurrence of ids[t] in the same batch row (-1 if none).  This turns the
dense [B*S, V] x [V, D] matmul (52 GFLOP + 102MB of W traffic) into:
  - a gather of the 1024 referenced W columns per batch row (dma_gather),
  - an O(L^2) pairwise-equality pass to get prev[] (vector engine),
  - a [S, L] x [L, D] masked matmul per batch row (134 MFLOP total).

Sharding: data-parallel over batch. 8 cores x 2 batch rows each. No
collectives; each core writes its own output slice.

Implementation notes (hard-won):
  * walrus codegen allows only ONE sync-wait per instruction; Bacc.compile()
    legalizes by splitting into EventSemaphore instructions.  Inputs that
    feed one TensorScalarPtr arrive via a single DMA to keep wait counts low.
  * tensor_tensor_reduce (extended-ISA) wedges the device on this runtime -
    use tensor_tensor + tensor_reduce instead.
  * dma_gather indices are int16: table rows must be <32768.  The two vocab
    halves are FOLDED into one [25000, 2*D] table; slot t fetches both
    candidate rows and the correct half is selected by splitting the mask
    matmul into lo/hi parts with host-provided half indicators.
  * dma_gather descriptor generation costs ~6.5us per 1024-slot gather and
    serializes across SWDGE queues; queue 0 additionally blocks the Pool
    sequencer for the duration.  So: only 2 gathers (folded table), on
    queues 1 and 2.  The Q7 ucode runs on a core pair per queue and BOTH
    cores stream the int16 index list: for queue q, rx reads partitions
    [32q, 32q+16) and tx reads [32q+16, 32q+32) - the list is duplicated in
    that window.  (CoreSim models only partitions 0-15 / queue 0.)
  * fp32 matmuls lower to 4 PE passes; gather table / mask / matmuls run in
    bf16 (mask is exact 0/1; PSUM accumulation stays fp32).  The DVE work
    runs in uint16/fp16 to hit the 4x/2x perf modes (all values <= 2048 are
    exact in fp16).
"""

import os
import sys

import numpy as np

for _p in ("/opt/trn_rl_repo", "/root/.axon_site/_ro/trn_rl_repo"):
    if os.path.isdir(_p) and _p not in sys.path:
        sys.path.append(_p)

import concourse.bacc as bacc
import concourse.bass as bass
import concourse.mybir as mybir
import concourse.tile as tile
from concourse.bass_utils import run_bass_kernel_spmd

P = 128          # partitions
B, S, L, V, D = 16, 64, 1024, 50000, 512
NCORES = 8
NB = B // NCORES     # batch rows per core = 2
NCH = L // P         # 128-position chunks per batch row = 8
VH = V // 2          # folded table rows = 25000
D2 = 2 * D           # folded row width = 1024
MVW = P * (NCH * (NCH + 1) // 2)   # total maskval width = 4608
MV1 = (1 + 2 + 3 + 4) * P          # first-half maskval width = 1280
CSTW = MVW + 2 * P + NCH           # mv | ij(256) | tpos(8) = 4872
IDSW = NB * L + NB * NCH           # idsb0 | idsb1 | idscf = 2064
NCI = L // 16        # idx columns per gather list = 64

AL = mybir.AluOpType
F32 = mybir.dt.float32
F16 = mybir.dt.float16
BF16 = mybir.dt.bfloat16
U16 = mybir.dt.uint16
I16 = mybir.dt.int16


def _build_program(sim_compat=False):
    nc = bacc.Bacc("TRN2", target_bir_lowering=False, debug=False,
                   num_devices=NCORES, num_swdge_queues=1 if sim_compat else 4)

    wt2 = nc.dram_tensor("wt2", [VH, D2], BF16, kind="ExternalInput").ap()
    idsall = nc.dram_tensor("idsall", [P, IDSW], U16, kind="ExternalInput").ap()
    # int16 gather lists, one [P, NCI] block per batch row; on HW the list
    # for gather g sits in partition rows [32*(g+1), 32*(g+2)) of its block
    idx16 = nc.dram_tensor("idx16", [P, NB * NCI], I16,
                           kind="ExternalInput").ap()
    cst = nc.dram_tensor("cst", [P, CSTW], F16, kind="ExternalInput").ap()
    # f32 per-partition scalar columns: tpos(8) | idscf(16) | hl(16)
    # (TensorScalarPtr requires f32 scalars for compare ops)
    cols = nc.dram_tensor("cols", [P, 8 + 2 * NB * NCH], F32,
                          kind="ExternalInput").ap()
    bias = nc.dram_tensor("bias", [D], F32, kind="ExternalInput").ap()
    out = nc.dram_tensor("out", [NB, S, D], F32, kind="ExternalOutput").ap()

    with tile.TileContext(nc) as tc:
        with (
            tc.tile_pool(name="const", bufs=1) as cp,
            tc.tile_pool(name="work", bufs=2) as wp,
            tc.tile_pool(name="psum", bufs=1, space="PSUM") as pp,
        ):
            # ---- input loads: idx first (gates gathers), split across the
            # two HWDGE rings (sync=SP, scalar=ACT) for parallel arrival
            idx_sb = cp.tile([P, NB * NCI], I16, tag="idx16")
            nc.sync.dma_start(out=idx_sb[:], in_=idx16)
            ids_sb = cp.tile([P, IDSW], U16, tag="idsall")
            nc.scalar.dma_start(out=ids_sb[:], in_=idsall)
            cols_sb = cp.tile([P, 8 + 2 * NB * NCH], F32, tag="cols")
            nc.scalar.dma_start(out=cols_sb[:], in_=cols)
            cst_sb = cp.tile([P, CSTW], F16, tag="cst")
            # split so ij/tpos and the first maskval blocks land early
            nc.sync.dma_start(out=cst_sb[:, MVW:], in_=cst[:, MVW:])
            nc.sync.dma_start(out=cst_sb[:, :MV1], in_=cst[:, :MV1])
            nc.sync.dma_start(out=cst_sb[:, MV1:MVW], in_=cst[:, MV1:MVW])
            bias_sb = cp.tile([1, D], F32, tag="bias")
            nc.scalar.dma_start(out=bias_sb[:], in_=bias.unsqueeze(0))
            ones_sb = cp.tile([1, S], F32, tag="ones")
            nc.vector.memset(ones_sb[:], 1.0)

            def idsb(b):          # [P, L] ids of batch row b, partition-bcast
                return ids_sb[:, b * L:(b + 1) * L]

            def idscf(k):         # [P, 1] f32 per-partition token col
                return cols_sb[:, 8 + k: 8 + k + 1]

            def hlcol(k):         # [P, 1] f32 hi-half indicator col
                return cols_sb[:, 8 + NB * NCH + k: 8 + NB * NCH + k + 1]

            ij_sb = cst_sb[:, MVW:MVW + 2 * P]          # [P, 256]
            tpos_sb = cols_sb[:, :NCH]                  # [P, 8] f32

            # ---- gather E2[t, :] = WT2[ids[t] % VH, :] (both halves),
            # slot t -> [t%128, t//128, :]; queues 1/2 (0 blocks the Pool NX)
            e_t = []
            for b in range(NB):
                eb = cp.tile([P, NCH * D2], BF16, tag=f"e{b}")
                nc.gpsimd.dma_gather(
                    eb[:].rearrange("p (c d) -> p c d", d=D2),
                    wt2,
                    idx_sb[:, b * NCI:(b + 1) * NCI],
                    L, L, D2,
                    queue_num=0 if sim_compat else b + 1)
                e_t.append(eb)

            # ---- prev1[t] = 1 + index of previous occurrence of ids[t] (0 if none)
            # chunk c holds t = c*128 + p in partition p, column c.
            prev1 = []
            for b in range(NB):
                pb = cp.tile([P, NCH], F32, tag=f"prev{b}")
                for c in range(NCH):
                    F = (c + 1) * P
                    off = (c * (c + 1) // 2) * P
                    eq = wp.tile([P, L], F16, tag="eq")
                    if b == 0 and c in (0, 4):
                        # absorb the maskval DMA-completion tick on a spare op
                        # (1-wait budget); WAW on eq orders it before the TS
                        src = 0 if c == 0 else MV1
                        nc.vector.tensor_tensor(out=eq[:1, :1],
                                                in0=cst_sb[:1, src:src + 1],
                                                in1=cst_sb[:1, src:src + 1],
                                                op=AL.add)
                    nc.vector.tensor_scalar(
                        out=eq[:, :F], in0=idsb(b)[:, :F],
                        scalar1=idscf(b * NCH + c), scalar2=None,
                        op0=AL.is_equal)
                    # NB: tensor_tensor_reduce would fuse these, but its
                    # extended-ISA ucode wedges the device on this runtime.
                    scr = wp.tile([P, L], F16, tag="scr")
                    nc.vector.tensor_tensor(
                        out=scr[:, :F], in0=eq[:, :F],
                        in1=cst_sb[:, off:off + F], op=AL.mult)
                    nc.vector.tensor_reduce(
                        out=pb[:, c:c + 1], in_=scr[:, :F],
                        axis=mybir.AxisListType.X, op=AL.max)
                prev1.append(pb)

            # ---- matmul accumulation: psum[b] = bias + sum_c Mt_c[b].T @ E_c[b]
            psums = []
            for b in range(NB):
                ps = pp.tile([S, D], F32, tag=f"ps{b}")
                nc.tensor.matmul(out=ps[:], lhsT=ones_sb[:], rhs=bias_sb[:],
                                 start=True, stop=False)
                psums.append(ps)

            # mask chunk Mt_c[p, f] for span f = b*64+s, position t = c*128+p:
            #   (i_f <= t) & (t < j_f) & (i_f >= prev1_b[t])
            # computed as (a - d) * g with a=[i<=t], d=[i<prev1] (d subset a),
            # g=[j>t]; then split into lo/hi-vocab-half masks for the folded
            # gather rows.
            for c in range(NCH):
                a_t = wp.tile([P, P], F16, tag="a")
                nc.vector.tensor_scalar(out=a_t[:], in0=ij_sb[:, :P],
                                        scalar1=tpos_sb[:, c:c + 1],
                                        scalar2=None, op0=AL.is_le)
                g_t = wp.tile([P, P], F16, tag="g")
                nc.vector.tensor_scalar(out=g_t[:], in0=ij_sb[:, P:2 * P],
                                        scalar1=tpos_sb[:, c:c + 1],
                                        scalar2=None, op0=AL.is_gt)
                d_t = wp.tile([P, P], F16, tag="d")
                for b in range(NB):
                    nc.vector.tensor_scalar(out=d_t[:, b * S:(b + 1) * S],
                                            in0=ij_sb[:, b * S:b * S + S],
                                            scalar1=prev1[b][:, c:c + 1],
                                            scalar2=None, op0=AL.is_lt)
                u_t = wp.tile([P, P], F16, tag="u")
                nc.vector.tensor_tensor(out=u_t[:], in0=a_t[:], in1=d_t[:],
                                        op=AL.subtract)
                m_t = wp.tile([P, P], BF16, tag="m")
                nc.vector.tensor_tensor(out=m_t[:], in0=u_t[:], in1=g_t[:],
                                        op=AL.mult)
                mlo = wp.tile([P, P], BF16, tag="mlo")
                mhi = wp.tile([P, P], BF16, tag="mhi")
                for b in range(NB):
                    sl = slice(b * S, (b + 1) * S)
                    nc.vector.tensor_scalar(out=mhi[:, sl], in0=m_t[:, sl],
                                            scalar1=hlcol(b * NCH + c),
                                            scalar2=None, op0=AL.mult)
                    nc.vector.tensor_tensor(out=mlo[:, sl], in0=m_t[:, sl],
                                            in1=mhi[:, sl], op=AL.subtract)
                for b in range(NB):
                    sl = slice(b * S, (b + 1) * S)
                    nc.tensor.matmul(out=psums[b][:],
                                     lhsT=mlo[:, sl],
                                     rhs=e_t[b][:, c * D2:c * D2 + D],
                                     start=False, stop=False)
                    nc.tensor.matmul(out=psums[b][:],
                                     lhsT=mhi[:, sl],
                                     rhs=e_t[b][:, c * D2 + D:(c + 1) * D2],
                                     start=False, stop=(c == NCH - 1))

            # ---- write out ----
            for b in range(NB):
                o_sb = wp.tile([S, D], F32, tag=f"o{b}")
                nc.vector.tensor_copy(out=o_sb[:], in_=psums[b][:])
                nc.sync.dma_start(out=out[b], in_=o_sb[:])

    # bacc passes: split excess sync waits into EventSemaphore insts,
    # move matmul waits to ldweights, populate extended-inst ISA bytes, etc.
    nc.compile()
    return nc


_NC_CACHE = {}


def _get_program(sim_compat=False):
    if sim_compat not in _NC_CACHE:
        _NC_CACHE[sim_compat] = _build_program(sim_compat)
    return _NC_CACHE[sim_compat]


def _host_constants():
    # maskval blocks: for chunk c (t = c*128+p), source positions f in
    # [0, (c+1)*128): value f+1 if f < t else -30000 (ignored by max;
    # fp16-exact).
    cstw = np.empty((P, CSTW), np.float16)
    f_idx = np.arange(L, dtype=np.float16)
    for c in range(NCH):
        F = (c + 1) * P
        off = (c * (c + 1) // 2) * P
        t = (c * P + np.arange(P, dtype=np.float16))[:, None]   # [P,1]
        cstw[:, off:off + F] = np.where(
            f_idx[None, :F].astype(np.float32) < t.astype(np.float32),
            (f_idx[None, :F].astype(np.float32) + 1.0),
            np.float32(-30000)).astype(np.float16)
    tpos = (np.arange(NCH, dtype=np.float32)[None, :] * P
            + np.arange(P, dtype=np.float32)[:, None])          # [P, NCH]
    cstw[:, MVW + 2 * P:] = tpos.astype(np.float16)
    return cstw


def _gather_list(ids_row):
    """int16 folded index list [16, NCI]: slot t at [t % 16, t // 16]."""
    return (ids_row % VH).reshape(NCI, 16).T.astype(np.int16)


def _make_in_maps(input_ids, span_idxs, W, b, sim_compat=False):
    import ml_dtypes
    ids = np.asarray(input_ids).astype(np.int64)        # [B, L]
    spans = np.asarray(span_idxs).astype(np.int64)      # [B, S, 2]
    Wf = np.asarray(W, dtype=np.float32)                # [D, V]
    WT = np.ascontiguousarray(Wf.T)                     # [V, D]
    # folded table: row v = [WT[v] | WT[v + VH]]
    wt2 = np.concatenate([WT[:VH], WT[VH:]], axis=1).astype(ml_dtypes.bfloat16)
    wt2 = np.ascontiguousarray(wt2)
    bf = np.ascontiguousarray(np.asarray(b, dtype=np.float32))  # [D]
    cst_base = _host_constants()

    in_maps = []
    for core in range(NCORES):
        sl = slice(NB * core, NB * (core + 1))
        ids_c = ids[sl]                                 # [NB, L]
        sp = spans[sl]                                  # [NB, S, 2]
        # column-chunk layout: [p, b*NCH + c] = ids_c[b, c*128 + p]
        idsc = ids_c.reshape(NB, NCH, P).transpose(2, 0, 1).reshape(P, NB * NCH)
        idsall = np.empty((P, IDSW), np.uint16)
        for bb in range(NB):
            idsall[:, bb * L:(bb + 1) * L] = ids_c[bb][None, :].astype(np.uint16)
        idsall[:, NB * L:] = idsc.astype(np.uint16)
        cols = np.empty((P, 8 + 2 * NB * NCH), np.float32)
        cols[:, :NCH] = (np.arange(NCH, dtype=np.float32)[None, :] * P
                         + np.arange(P, dtype=np.float32)[:, None])
        cols[:, NCH:NCH + NB * NCH] = idsc.astype(np.float32)
        cols[:, NCH + NB * NCH:] = (idsc >= VH).astype(np.float32)
        idx16 = np.zeros((P, NB * NCI), np.int16)
        for bb in range(NB):
            lst = _gather_list(ids_c[bb])               # [16, NCI]
            base = 0 if sim_compat else 32 * (bb + 1)
            idx16[base:base + 16, bb * NCI:(bb + 1) * NCI] = lst
            idx16[base + 16:base + 32, bb * NCI:(bb + 1) * NCI] = lst
        cst = cst_base.copy()
        ij = np.concatenate([sp[..., 0].reshape(-1),
                             sp[..., 1].reshape(-1)]).astype(np.float16)
        cst[:, MVW:MVW + 2 * P] = ij[None, :]
        in_maps.append({
            "wt2": wt2,
            "idsall": idsall,
            "idx16": np.ascontiguousarray(idx16),
            "cst": cst,
            "cols": np.ascontiguousarray(cols),
            "bias": bf,
        })
    return in_maps


def run(input_ids, span_idxs, W, b, trace=False, **spmd_kwargs):
    """Build + run on 8 cores; returns (out [B,S,D] f32, BassKernelResults)."""
    nc = _get_program()
    in_maps = _make_in_maps(input_ids, span_idxs, W, b)
    res = run_bass_kernel_spmd(nc, in_maps, list(range(NCORES)),
                               trace=trace, **spmd_kwargs)
    outs = [res.results[i]["out"] for i in range(NCORES)]
    full = np.concatenate(outs, axis=0).reshape(B, S, D).astype(np.float32)
    return full, res


def kernel(input_ids, span_idxs, W, b):
    out, _ = run(input_ids, span_idxs, W, b)
    return out
